# revision 77
# baseline (speedup 1.0000x reference)
"""Distributed attention kernel for Trainium2 (8 NeuronCores).

Problem: non-causal multi-head attention with GQA (16 q heads, 4 kv heads,
head_dim 64, dim 1024, batch 2, seqlen 2048), fp32.

Sharding (per the batch+head hint): core c in 0..7 handles batch b = c//4
and kv-head-group g = c%4 (q heads 4g..4g+3, kv head g). Each core holds the
full sequence, so softmax needs no communication. The output projection is
row-parallel: core (b, g) computes the partial product
O_g @ wo[256g:256(g+1), :] and the host sums the 4 partials per batch
(the gather/unshard step).

Per-core dataflow (v2 — PV restructured to seq-major output):
  xT = x[b].T                               (1024, S) fed from host, bf16
  QT = wq_g.T @ xT                          (256, S)  f32r [head pair ft:
                                              rows 0-63 head 2ft, 64-127 2ft+1]
  KVT = [wk_g | wv_g].T @ xT                (128, S)  f32r [K^T ; V^T]
  K^T duplicated to partitions 64-127 (gpsimd DMA) so both heads of a pair
  run score matmuls from disjoint partition ranges.
  V transposed per 128-k tile (PE) and packed seq-major with a ones column:
  v1[kt] = [V_kt | 1]  (128, 65) bf16.
  Per (qc of 512 q, ft head-pair):
    per kt: S^T = K^T.T @ Q^T -> psum [128, 2, 512]; one exp (ScalarE)
            -> e2t[:, :, kt, :] bf16 (slab for the whole phase).
    PV with the probabilities STATIONARY: out[q, d] += e2^T @ [V|1]
    accumulated qt-major into [128, 4, 65] psum (sequential sub-bank
    accumulation groups; hardware allows only one OPEN group per bank).
    Cost: 65 cols/moving pass instead of 512 -> halves PV PE time.
    Normalize: DVE reciprocal of the ones-column sum, GPSIMD per-partition
    multiply -> o bf16 [q, 2*64]; XBAR dma transpose -> ot feature-major.
  out rows = OT.T @ wo_g (PE), staged bf16, DMA out. Host sums partials.

Engine budget (cost model): ACT exp 128x~1.04us = 133us (bound), PE ~287K
cyc = 120us, DVE ~43us, Pool ~12us, DMA ~28us.
"""

import numpy as np
from contextlib import ExitStack

import concourse.bass as bass
import concourse.mybir as mybir
import concourse.tile as tile
from concourse.bass import ds
from concourse import bass_utils
from concourse.alu_op_type import AluOpType

F32 = mybir.dt.float32
F32R = mybir.dt.float32r
BF16 = mybir.dt.bfloat16

DIM = 1024
N_HEADS = 16
N_KV_HEADS = 4
HD = 64
FH = 256                   # q features per core (4 heads x 64)
KV = 128                   # [K | V] projected feature width per core
D_TILES = DIM // 128       # 8
SEQ = 2048
BSZ = 2
N_CORES = 8


def build_attention_core(nc, S=SEQ, use_f32r=True, debug_taps=False):
    """Emit the per-core kernel. S = sequence length (multiple of 512)."""
    QCH = 512                  # q-chunk width (psum bank = 512 f32)
    S_TILES = S // 128         # k tiles
    N_QC = S // QCH            # q chunks
    MDT = F32R if use_f32r else F32

    xT = nc.declare_dram_parameter("xT", [DIM, S], BF16, isOutput=False)
    # weights arrive host-pre-tiled (partition-major) so each DMA moves
    # 2-4KB contiguous runs per partition instead of 256-512B rows
    wq = nc.declare_dram_parameter("wq", [128, D_TILES, FH], BF16, isOutput=False)
    wkv = nc.declare_dram_parameter("wkv", [128, D_TILES, KV], BF16, isOutput=False)
    wo = nc.declare_dram_parameter("wo", [128, 2, DIM], BF16, isOutput=False)
    out = nc.declare_dram_parameter("out", [S, DIM], BF16, isOutput=True)
    if debug_taps:
        dbg_qt = nc.declare_dram_parameter("dbg_qt", [128, 2, 512], F32, isOutput=True)
        dbg_kv = nc.declare_dram_parameter("dbg_kv", [128, 512], F32, isOutput=True)
        dbg_kt2 = nc.declare_dram_parameter("dbg_kt2", [128, 512], F32, isOutput=True)
        dbg_ot = nc.declare_dram_parameter("dbg_ot", [128, 2, 512], BF16, isOutput=True)

    with tile.TileContext(nc) as tc:
      with ExitStack() as ctx:
        const_p = ctx.enter_context(tc.tile_pool(name="const", bufs=1))
        big_p = ctx.enter_context(tc.tile_pool(name="big", bufs=1))
        e2_p = ctx.enter_context(tc.tile_pool(name="e2", bufs=2))
        osb_p = ctx.enter_context(tc.tile_pool(name="osb", bufs=2))
        rz_p = ctx.enter_context(tc.tile_pool(name="rz", bufs=8))
        stg_p = ctx.enter_context(tc.tile_pool(name="stg", bufs=4))
        ps_sc = ctx.enter_context(tc.tile_pool(name="ps_sc", bufs=2, space="PSUM"))
        ps_acc = ctx.enter_context(tc.tile_pool(name="ps_acc", bufs=2, space="PSUM"))

        # Warm the PE clock (p-state ramp) from a memset tile so warmup
        # starts immediately instead of waiting on any DMA; also preload
        # the exp table (a real-hardware cost the model does not charge).
        wsrc = const_p.tile([128, 512], BF16)
        nc.vector.memset(wsrc[:], 1.0)
        warm = const_p.tile([128, 8], F32)
        nc.scalar.activation(
            warm[0:1, 0:1], wsrc[0:1, 0:2].bitcast(F32),
            mybir.ActivationFunctionType.Exp,
        )
        # ramp: short mms to mid p-state, then 512-wide to span the 3us
        # full-clock threshold and bridge until the first x chunk lands.
        warmps = ps_sc.tile([128, 3, QCH], F32, tag="sc")
        for w in range(4):
            nc.tensor.matmul(
                warmps[:, 0, 0:128], wsrc[:, 0:128], wsrc[:, 0:128],
                start=(w == 0), stop=False,
            )
        for w in range(10):
            nc.tensor.matmul(
                warmps[:, 0, :], wsrc[:, 0:128], wsrc[:],
                start=False, stop=(w == 9),
            )
        # identity matrices generated on-device (diagonal select over ones)
        # so no DMA sits on the transposes' critical path
        onesr = const_p.tile([128, 128], F32)
        nc.vector.memset(onesr[:], 1.0)
        ident = const_p.tile([128, 128], MDT)
        nc.gpsimd.affine_select(
            ident[:], onesr[:], [[1, 128]], AluOpType.is_equal, 0.0,
            base=0, channel_multiplier=-1,
        )
        identb = const_p.tile([128, 128], BF16)
        nc.gpsimd.affine_select(
            identb[:], wsrc[:, 0:128], [[1, 128]], AluOpType.is_equal, 0.0,
            base=0, channel_multiplier=-1,
        )

        # ---- projections -------------------------------------------------
        kvt_sb = big_p.tile([128, S], MDT)
        kt2_sb = big_p.tile([128, S], MDT)
        qt_sb = big_p.tile([128, 2, S], MDT)
        v1_sb = big_p.tile([128, S_TILES, 65], BF16)
        nc.vector.memset(v1_sb[:, :, 64:65], 1.0)
        ot_sb = big_p.tile([128, 2, S], BF16)

        # ---- load inputs, ordered so kvproj chunks land just-in-time -----
        wq_sb = big_p.tile([128, D_TILES, FH], BF16)
        wkv_sb = big_p.tile([128, D_TILES, KV], BF16)
        xt_sb = big_p.tile([128, D_TILES, S], BF16)
        nc.sync.dma_start(wkv_sb[:, :, :], wkv[:, :, :])

        def xchunk(sc, half):
            # batched half-chunk load: 4 d-tiles in one DMA (one HWDGE issue)
            a0 = half * 4
            nc.sync.dma_start(
                xt_sb[:, ds(a0, 4), ds(sc * QCH, QCH)],
                xT[ds(a0 * 128, 512), ds(sc * QCH, QCH)].rearrange(
                    "(a p) q -> p a q", p=128
                ),
            )

        xchunk(0, 0)
        xchunk(0, 1)
        nc.sync.dma_start(wq_sb[:, :, :], wq[:, :, :])
        wo_sb = big_p.tile([128, 2, DIM], BF16)

        def kt2dma(sc):
            # K^T partition-shift (rows 0:64 -> 64:128) on the SP queue,
            # emitted right AFTER kvproj(sc) so the data dependency is real;
            # its wait paces the following x-chunk issues so small critical
            # transfers are not starved by the big x stream.
            nc.sync.dma_start(
                kt2_sb[64:128, ds(sc * QCH, QCH)],
                kvt_sb[0:64, ds(sc * QCH, QCH)],
            )

        def kvproj(sc, split=False):
            acc = ps_acc.tile([128, QCH], F32, tag="acc", name="kvacc")
            for a in range(D_TILES):
                nc.tensor.matmul(
                    acc[:],
                    wkv_sb[:, a, :],
                    xt_sb[:, a, ds(sc * QCH, QCH)],
                    start=(a == 0),
                    stop=(a == D_TILES - 1),
                )
            if split:
                # K rows first: kt2dma (and the x stream it paces) unblocks
                # before the V half lands
                nc.vector.tensor_copy(kvt_sb[0:64, ds(sc * QCH, QCH)],
                                      acc[0:64, :])
                nc.vector.tensor_copy(kvt_sb[64:128, ds(sc * QCH, QCH)],
                                      acc[64:128, :])
            else:
                nc.vector.tensor_copy(kvt_sb[:, ds(sc * QCH, QCH)], acc[:])

        def qproj(sc, fts=(0, 1)):
            for ft in fts:
                acc = ps_acc.tile([128, QCH], F32, tag="acc", name="qacc")
                for a in range(D_TILES):
                    nc.tensor.matmul(
                        acc[:],
                        wq_sb[:, a, ds(ft * 128, 128)],
                        xt_sb[:, a, ds(sc * QCH, QCH)],
                        start=(a == 0),
                        stop=(a == D_TILES - 1),
                    )
                nc.vector.tensor_copy(qt_sb[:, ft, ds(sc * QCH, QCH)], acc[:])

        def vprep(sc):
            # V^T rows of kvt (64:128) -> seq-major v1 tiles (bf16, ones col)
            for kt in range(4 * sc, 4 * sc + 4):
                tr = ps_acc.tile([128, QCH], F32, tag="acc", name="tracc")
                nc.tensor.transpose(
                    tr[:, 0:64].bitcast(MDT), kvt_sb[64:128, ds(kt * 128, 128)],
                    ident[64:128, 64:128],
                )
                nc.vector.tensor_copy(v1_sb[:, kt, 0:64], tr[:, 0:64].bitcast(MDT))

        kvproj(0, split=True)
        kt2dma(0)
        xchunk(1, 0)
        xchunk(1, 1)
        qproj(0, fts=(0,))

        # ---- deferred-work generators -----------------------------------
        def outproj_st(qc, st, stage_eng="dve"):
            """One 128-row block of the output projection: two psum accs
            staged into a single [128, 1024] store (one HWDGE issue).
            stage_eng picks who drains psum: DVE in steady state, ACT in
            the tail (where the exp stream is finished and ACT is idle)."""
            row0 = qc * QCH + st * 128
            stg = stg_p.tile([128, DIM], BF16, tag="ostg", name="stg")
            for c in range(2):
                acc = ps_acc.tile([128, 512], F32, tag="acc", name="oacc")
                for t in range(2):
                    nc.tensor.matmul(
                        acc[:],
                        ot_sb[:, t, ds(row0, 128)],
                        wo_sb[:, t, ds(c * 512, 512)],
                        start=(t == 0),
                        stop=(t == 1),
                    )
                if stage_eng == "act" and c == 0:
                    nc.scalar.copy(stg[:, ds(c * 512, 512)], acc[:])
                else:
                    nc.vector.tensor_copy(stg[:, ds(c * 512, 512)], acc[:])
                yield
            nc.sync.dma_start(out[ds(row0, 128), :], stg[:])

        def outproj_gen(qc):
            for st in range(QCH // 128):
                yield from outproj_st(qc, st)

        def pv_work(qc, ft, e2t, tail=False):
            """PV + normalize + transpose for phase (qc, ft); yields between
            slices so the caller can interleave it with the next phase.
            Each (query-block, head) accumulation bursts through one acc-pool
            bank (open-close within the burst; one open group per bank) and
            is normalized straight from PSUM, freeing the bank immediately.
            e2t is block-major: block g = 2*kt + h."""
            o_sb = osb_p.tile([128, 4, 128], BF16, tag="osb", name="osb")

            def group_body(h, qt4, split, pool=None):
                pvb = (ps_sc.tile([128, 3, QCH], F32, tag="sc", name="pvsc")[:, 0, :]
                       if pool == "sc" else
                       ps_acc.tile([128, 512], F32, tag="acc", name="pvb"))
                for kt in range(S_TILES):
                    nc.tensor.matmul(
                        pvb[:, 0:65],
                        e2t[:, 2 * kt + h, ds(qt4 * 128, 128)],
                        v1_sb[:, kt, :],
                        start=(kt == 0),
                        stop=(kt == S_TILES - 1),
                    )
                    if split and kt == S_TILES // 2 - 1:
                        yield
                rz = rz_p.tile([128, 1], F32, tag="rz", name="rz")
                nc.vector.reciprocal(rz[:], pvb[:, 64:65])
                nc.vector.tensor_scalar(
                    o_sb[:, qt4, ds(h * 64, 64)], pvb[:, 0:64],
                    rz[:], None, AluOpType.mult,
                )
                if split:
                    yield

            def group(h, qt4):
                for _ in group_body(h, qt4, False):
                    pass

            def group_split(h, qt4):
                yield from group_body(h, qt4, True)

            def transp(qt4, pool):
                tr = (ps_sc.tile([128, 3, QCH], F32, tag="sc", name="trsc")
                      if pool == "sc" else
                      ps_acc.tile([128, 512], F32, tag="acc", name="trx"))
                trv = (tr[:, 0, :] if pool == "sc" else tr[:, :]).bitcast(BF16)
                nc.tensor.transpose(trv[:, 0:128], o_sb[:, qt4, :], identb[:, :])
                nc.vector.tensor_copy(
                    ot_sb[:, ft, ds(qc * QCH + qt4 * 128, 128)], trv[:, 0:128]
                )

            if tail:
                for qt4 in range(4):
                    for h in range(2):
                        # qt0 bursts through the acc banks so they chase the
                        # final exps concurrently with the sc pool's last
                        # score tiles; later blocks use the freed sc pool.
                        for _ in group_body(h, qt4, False,
                                            pool=None if qt4 == 0 else "sc"):
                            pass
                    # chase each block immediately: PE transpose (into the
                    # now-idle score pool) then its output-projection rows,
                    # staged on the idle ACT engine.
                    transp(qt4, "sc")
                    for _ in outproj_st(qc, qt4, stage_eng="act"):
                        pass
                return
            for qt4 in range(4):
                for h in range(2):
                    yield from group_split(h, qt4)
                transp(qt4, "acc")
                yield

        # ---- attention phases -------------------------------------------
        prev_gen = None
        for qc in range(N_QC):
            qsl = ds(qc * QCH, QCH)
            for ft in range(2):
                e2t = e2_p.tile([128, 2 * S_TILES, QCH], BF16, tag="e2t",
                                name="e2t")
                # deferred work interleaved into this phase's kt slots:
                # previous phase's PV chain, this-column qproj, the output
                # projection two phases back (at ft==1), and (first phase)
                # the streaming K/V projections.
                slotmap = {}
                steps = []
                if qc == 0 and ft == 0:
                    # streaming prologue: K/V projections placed just before
                    # the first score tile that needs them, so early scores
                    # are not head-of-line blocked behind later x chunks.
                    def kvstage(c):
                        kvproj(c)
                        kt2dma(c)
                        if c + 1 < N_QC:
                            xchunk(c + 1, 0)
                            xchunk(c + 1, 1)
                        else:
                            nc.sync.dma_start(wo_sb[:, :, :], wo[:, :, :])

                    def kvstage_late(c):
                        # logical timestamp visible only to the Tile
                        # scheduler's internal sim: stops it hoisting this
                        # projection ahead of the first score tiles (its x
                        # chunk arrives much later than that sim believes).
                        with tc.tile_wait_until(0.05 * c):
                            kvstage(c)

                    slotmap = {
                        3: lambda: kvstage_late(1),
                        5: lambda: vprep(0),
                        6: lambda: vprep(1),
                        7: lambda: kvstage_late(2),
                        9: lambda: vprep(2),
                        11: lambda: kvstage_late(3),
                        12: lambda: vprep(3),
                        13: lambda: qproj(0, fts=(1,)),
                        15: lambda: qproj(1, fts=(0,)),
                    }
                elif qc + 1 < N_QC:
                    _qh = {}

                    def qp_half(first, qc=qc, ft=ft, _qh=_qh):
                        sc_n = qc + 1
                        if first:
                            _qh["acc"] = ps_acc.tile(
                                [128, QCH], F32, tag="acc", name="qacc"
                            )
                        acc = _qh["acc"]
                        for a in (range(4) if first else range(4, D_TILES)):
                            nc.tensor.matmul(
                                acc[:],
                                wq_sb[:, a, ds(ft * 128, 128)],
                                xt_sb[:, a, ds(sc_n * QCH, QCH)],
                                start=(a == 0),
                                stop=(a == D_TILES - 1),
                            )
                        if not first:
                            nc.vector.tensor_copy(
                                qt_sb[:, ft, ds(sc_n * QCH, QCH)], acc[:]
                            )

                    steps.append(lambda: qp_half(True))
                    steps.append(lambda: qp_half(False))
                gens = []
                if ft == 1 and qc >= 1:
                    gens.append(outproj_gen(qc - 1))
                if prev_gen is not None:
                    gens.append(prev_gen)
                rr = [0]

                def drain_one():
                    # round-robin across active generators
                    while gens:
                        i = rr[0] % len(gens)
                        try:
                            next(gens[i])
                            rr[0] += 1
                            return
                        except StopIteration:
                            gens.pop(i)

                sc3 = None
                for kt in range(S_TILES):
                    ksl = ds(kt * 128, 128)
                    for h in range(2):
                        g = 2 * kt + h
                        slot = g % 3
                        if slot == 0:
                            sc3 = ps_sc.tile([128, 3, QCH], F32, tag="sc",
                                             name="sc3")
                        if h == 0:
                            nc.tensor.matmul(
                                sc3[:, slot, :], kvt_sb[0:64, ksl],
                                qt_sb[0:64, ft, qsl],
                                start=True, stop=True,
                            )
                        else:
                            nc.tensor.matmul(
                                sc3[:, slot, :], kt2_sb[64:128, ksl],
                                qt_sb[64:128, ft, qsl],
                                start=True, stop=True,
                            )
                        if slot == 2 or g == 2 * S_TILES - 1:
                            n = slot + 1
                            nc.scalar.activation(
                                e2t[:, ds(g - n + 1, n), :], sc3[:, 0:n, :],
                                mybir.ActivationFunctionType.Exp,
                            )
                    if kt in slotmap:
                        slotmap[kt]()
                    elif steps:
                        steps.pop(0)()
                    else:
                        drain_one()
                while gens:
                    drain_one()

                last = (qc == N_QC - 1 and ft == 1)
                prev_gen = pv_work(qc, ft, e2t, tail=last)
        # tail: final phase's PV chain + its output projection
        for _ in prev_gen:
            pass
        if debug_taps:
            nc.sync.dma_start(dbg_qt[:, :, :], qt_sb[:, :, 0:512].bitcast(F32))
            nc.sync.dma_start(dbg_kv[:, :], kvt_sb[:, 0:512].bitcast(F32))
            nc.sync.dma_start(dbg_kt2[:, :], kt2_sb[:, 0:512].bitcast(F32))
            nc.sync.dma_start(dbg_ot[:, :, :], ot_sb[:, :, 0:512])

    return nc


# The neuronx compiler in this environment accepts only ONE sync-wait command
# per instruction; Tile emits instructions with several. Waiting is monotone,
# so hoisting all but the last wait onto same-engine NoOps is equivalent.
_wsctr = [0]


def split_multi_waits(nc):
    n_split = 0
    for f in nc.m.functions:
        for bb in f.blocks:
            insts = bb.instructions
            if not any(
                i.sync_info is not None and len(i.sync_info.on_wait) > 1
                for i in insts
            ):
                continue
            new = []
            for i in insts:
                si = i.sync_info
                if si is not None and len(si.on_wait) > 1:
                    waits = list(si.on_wait)
                    for w in waits[:-1]:
                        _wsctr[0] += 1
                        nop = mybir.InstNoOp(name=f"wsplit_{_wsctr[0]}", ins=[], outs=[])
                        nop.engine = i.engine
                        nop.sync_info = mybir.SyncInfo(on_wait=[w], on_update=[])
                        new.append(nop)
                    i.sync_info = mybir.SyncInfo(
                        on_wait=[waits[-1]], on_update=list(si.on_update)
                    )
                    n_split += 1
                new.append(i)
            bb.instructions = new
    return n_split


def build(use_f32r=True):
    nc = bass.Bass(target_bir_lowering=False)
    build_attention_core(nc, SEQ, use_f32r=use_f32r)
    split_multi_waits(nc)
    return nc


def shard_inputs(x, wq, wk, wv, wo):
    """Full inputs -> per-core in_maps. Core c = (b = c//4, g = c%4)."""
    x = np.asarray(x, np.float32)
    wq = np.asarray(wq, np.float32)
    wk = np.asarray(wk, np.float32)
    wv = np.asarray(wv, np.float32)
    wo = np.asarray(wo, np.float32)
    import ml_dtypes
    bf16 = ml_dtypes.bfloat16
    xTs = [np.ascontiguousarray(x[b].T).astype(bf16) for b in range(BSZ)]

    def tile_pmajor(w, nt):
        # [nt*128, n] -> [128, nt, n] (partition-major, contiguous rows)
        n = w.shape[1]
        return np.ascontiguousarray(
            w.reshape(nt, 128, n).transpose(1, 0, 2)
        ).astype(bf16)

    in_maps = []
    for c in range(N_CORES):
        b, g = c // 4, c % 4
        # fold the 1/sqrt(head_dim) score scaling into wq
        wq_g = tile_pmajor(wq[:, g * FH:(g + 1) * FH] * (1.0 / np.sqrt(HD)), 8)
        wkv_g = tile_pmajor(
            np.concatenate(
                [wk[:, g * HD:(g + 1) * HD], wv[:, g * HD:(g + 1) * HD]], axis=1
            ), 8,
        )
        wo_g = tile_pmajor(wo[g * FH:(g + 1) * FH, :], 2)
        in_maps.append(
            {"xT": xTs[b], "wq": wq_g, "wkv": wkv_g, "wo": wo_g}
        )
    return in_maps


def unshard_output(results):
    """Sum the 4 row-parallel partial outputs per batch."""
    out = np.zeros((BSZ, SEQ, DIM), np.float32)
    for c in range(N_CORES):
        out[c // 4] += np.asarray(results[c]["out"], np.float32)
    return out


_cache = {}


def kernel(x, wq, wk, wv, wo):
    if "nc" not in _cache:
        _cache["nc"] = build()
    nc = _cache["nc"]
    in_maps = shard_inputs(x, wq, wk, wv, wo)
    try:
        res = bass_utils.run_bass_kernel_spmd(
            nc, in_maps, core_ids=list(range(N_CORES))
        )
    except ModuleNotFoundError:
        # BASS_TRACE under an axon client without the NTFF hook module;
        # rerun untraced.
        import os

        os.environ["BASS_NEVER_TRACE"] = "1"
        res = bass_utils.run_bass_kernel_spmd(
            nc, in_maps, core_ids=list(range(N_CORES))
        )
    return unshard_output(res.results)


# revision 78
# speedup vs baseline: 1.0519x; 1.0519x over previous
"""Distributed attention kernel for Trainium2 (8 NeuronCores).

Problem: non-causal multi-head attention with GQA (16 q heads, 4 kv heads,
head_dim 64, dim 1024, batch 2, seqlen 2048), fp32.

Sharding (per the batch+head hint): core c in 0..7 handles batch b = c//4
and kv-head-group g = c%4 (q heads 4g..4g+3, kv head g). Each core holds the
full sequence, so softmax needs no communication. The output projection is
row-parallel: core (b, g) computes the partial product
O_g @ wo[256g:256(g+1), :] and the host sums the 4 partials per batch
(the gather/unshard step).

Per-core dataflow (v2 — PV restructured to seq-major output):
  xT = x[b].T                               (1024, S) fed from host, bf16
  QT = wq_g.T @ xT                          (256, S)  f32r [head pair ft:
                                              rows 0-63 head 2ft, 64-127 2ft+1]
  KVT = [wk_g | wv_g].T @ xT                (128, S)  f32r [K^T ; V^T]
  K^T duplicated to partitions 64-127 (gpsimd DMA) so both heads of a pair
  run score matmuls from disjoint partition ranges.
  V transposed per 128-k tile (PE) and packed seq-major with a ones column:
  v1[kt] = [V_kt | 1]  (128, 65) bf16.
  Per (qc of 512 q, ft head-pair):
    per kt: S^T = K^T.T @ Q^T -> psum [128, 2, 512]; one exp (ScalarE)
            -> e2t[:, :, kt, :] bf16 (slab for the whole phase).
    PV with the probabilities STATIONARY: out[q, d] += e2^T @ [V|1]
    accumulated qt-major into [128, 4, 65] psum (sequential sub-bank
    accumulation groups; hardware allows only one OPEN group per bank).
    Cost: 65 cols/moving pass instead of 512 -> halves PV PE time.
    Normalize: DVE reciprocal of the ones-column sum, GPSIMD per-partition
    multiply -> o bf16 [q, 2*64]; XBAR dma transpose -> ot feature-major.
  out rows = OT.T @ wo_g (PE), staged bf16, DMA out. Host sums partials.

Engine budget (cost model): ACT exp 128x~1.04us = 133us (bound), PE ~287K
cyc = 120us, DVE ~43us, Pool ~12us, DMA ~28us.
"""

import numpy as np
from contextlib import ExitStack

import concourse.bass as bass
import concourse.mybir as mybir
import concourse.tile as tile
from concourse.bass import ds
from concourse import bass_utils
from concourse.alu_op_type import AluOpType

F32 = mybir.dt.float32
F32R = mybir.dt.float32r
BF16 = mybir.dt.bfloat16

DIM = 1024
N_HEADS = 16
N_KV_HEADS = 4
HD = 64
FH = 256                   # q features per core (4 heads x 64)
KV = 128                   # [K | V] projected feature width per core
D_TILES = DIM // 128       # 8
SEQ = 2048
BSZ = 2
N_CORES = 8


def build_attention_core(nc, S=SEQ, use_f32r=True, debug_taps=False):
    """Emit the per-core kernel. S = sequence length (multiple of 512)."""
    QCH = 512                  # q-chunk width (psum bank = 512 f32)
    S_TILES = S // 128         # k tiles
    N_QC = S // QCH            # q chunks
    MDT = F32R if use_f32r else F32

    xT = nc.declare_dram_parameter("xT", [DIM, S], BF16, isOutput=False)
    # weights arrive host-pre-tiled (partition-major) so each DMA moves
    # 2-4KB contiguous runs per partition instead of 256-512B rows
    wq = nc.declare_dram_parameter("wq", [128, D_TILES, FH], BF16, isOutput=False)
    wkv = nc.declare_dram_parameter("wkv", [128, D_TILES, KV], BF16, isOutput=False)
    wo = nc.declare_dram_parameter("wo", [128, 2, DIM], BF16, isOutput=False)
    out = nc.declare_dram_parameter("out", [S, DIM], BF16, isOutput=True)
    if debug_taps:
        dbg_qt = nc.declare_dram_parameter("dbg_qt", [128, 2, 512], F32, isOutput=True)
        dbg_kv = nc.declare_dram_parameter("dbg_kv", [128, 512], F32, isOutput=True)
        dbg_kt2 = nc.declare_dram_parameter("dbg_kt2", [128, 512], F32, isOutput=True)
        dbg_ot = nc.declare_dram_parameter("dbg_ot", [128, 2, 512], BF16, isOutput=True)

    with tile.TileContext(nc) as tc:
      with ExitStack() as ctx:
        const_p = ctx.enter_context(tc.tile_pool(name="const", bufs=1))
        big_p = ctx.enter_context(tc.tile_pool(name="big", bufs=1))
        e2_p = ctx.enter_context(tc.tile_pool(name="e2", bufs=2))
        osb_p = ctx.enter_context(tc.tile_pool(name="osb", bufs=2))
        rz_p = ctx.enter_context(tc.tile_pool(name="rz", bufs=8))
        stg_p = ctx.enter_context(tc.tile_pool(name="stg", bufs=4))
        ps_sc = ctx.enter_context(tc.tile_pool(name="ps_sc", bufs=2, space="PSUM"))
        ps_acc = ctx.enter_context(tc.tile_pool(name="ps_acc", bufs=2, space="PSUM"))

        # Warm the PE clock (p-state ramp) from a memset tile so warmup
        # starts immediately instead of waiting on any DMA; also preload
        # the exp table (a real-hardware cost the model does not charge).
        wsrc = const_p.tile([128, 512], BF16)
        nc.vector.memset(wsrc[:], 1.0)
        warm = const_p.tile([128, 8], F32)
        nc.scalar.activation(
            warm[0:1, 0:1], wsrc[0:1, 0:2].bitcast(F32),
            mybir.ActivationFunctionType.Exp,
        )
        # ramp: short mms to mid p-state, then 512-wide to span the 3us
        # full-clock threshold and bridge until the first x chunk lands.
        warmps = ps_sc.tile([128, 3, QCH], F32, tag="sc")
        for w in range(4):
            nc.tensor.matmul(
                warmps[:, 0, 0:128], wsrc[:, 0:128], wsrc[:, 0:128],
                start=(w == 0), stop=False,
            )
        for w in range(10):
            nc.tensor.matmul(
                warmps[:, 0, :], wsrc[:, 0:128], wsrc[:],
                start=False, stop=(w == 9),
            )
        # identity matrices generated on-device (diagonal select over ones)
        # so no DMA sits on the transposes' critical path
        onesr = const_p.tile([128, 128], F32)
        nc.vector.memset(onesr[:], 1.0)
        ident = const_p.tile([128, 128], MDT)
        nc.gpsimd.affine_select(
            ident[:], onesr[:], [[1, 128]], AluOpType.is_equal, 0.0,
            base=0, channel_multiplier=-1,
        )
        identb = const_p.tile([128, 128], BF16)
        nc.gpsimd.affine_select(
            identb[:], wsrc[:, 0:128], [[1, 128]], AluOpType.is_equal, 0.0,
            base=0, channel_multiplier=-1,
        )

        # ---- projections -------------------------------------------------
        kvt_sb = big_p.tile([128, S], MDT)
        kt2_sb = big_p.tile([128, S], MDT)
        qt_sb = big_p.tile([128, 2, S], MDT)
        v1_sb = big_p.tile([128, S_TILES, 65], BF16)
        nc.vector.memset(v1_sb[:, :, 64:65], 1.0)
        ot_sb = big_p.tile([128, 2, S], BF16)

        # ---- load inputs, ordered so kvproj chunks land just-in-time -----
        wq_sb = big_p.tile([128, D_TILES, FH], BF16)
        wkv_sb = big_p.tile([128, D_TILES, KV], BF16)
        xt_sb = big_p.tile([128, D_TILES, S], BF16)
        nc.sync.dma_start(wkv_sb[:, :, :], wkv[:, :, :])

        def xchunk(sc, half):
            # batched half-chunk load: 4 d-tiles in one DMA (one HWDGE issue)
            a0 = half * 4
            nc.sync.dma_start(
                xt_sb[:, ds(a0, 4), ds(sc * QCH, QCH)],
                xT[ds(a0 * 128, 512), ds(sc * QCH, QCH)].rearrange(
                    "(a p) q -> p a q", p=128
                ),
            )

        xchunk(0, 0)
        xchunk(0, 1)
        nc.sync.dma_start(wq_sb[:, :, :], wq[:, :, :])
        wo_sb = big_p.tile([128, 2, DIM], BF16)

        def kt2dma(sc):
            # K^T partition-shift (rows 0:64 -> 64:128) on the SP queue,
            # emitted right AFTER kvproj(sc) so the data dependency is real;
            # its wait paces the following x-chunk issues so small critical
            # transfers are not starved by the big x stream.
            nc.sync.dma_start(
                kt2_sb[64:128, ds(sc * QCH, QCH)],
                kvt_sb[0:64, ds(sc * QCH, QCH)],
            )

        def kvproj(sc, split=False):
            acc = ps_acc.tile([128, QCH], F32, tag="acc", name="kvacc")
            for a in range(D_TILES):
                nc.tensor.matmul(
                    acc[:],
                    wkv_sb[:, a, :],
                    xt_sb[:, a, ds(sc * QCH, QCH)],
                    start=(a == 0),
                    stop=(a == D_TILES - 1),
                )
            if split:
                # K rows first: kt2dma (and the x stream it paces) unblocks
                # before the V half lands
                nc.vector.tensor_copy(kvt_sb[0:64, ds(sc * QCH, QCH)],
                                      acc[0:64, :])
                nc.vector.tensor_copy(kvt_sb[64:128, ds(sc * QCH, QCH)],
                                      acc[64:128, :])
            else:
                nc.vector.tensor_copy(kvt_sb[:, ds(sc * QCH, QCH)], acc[:])

        def qproj(sc, fts=(0, 1)):
            for ft in fts:
                acc = ps_acc.tile([128, QCH], F32, tag="acc", name="qacc")
                for a in range(D_TILES):
                    nc.tensor.matmul(
                        acc[:],
                        wq_sb[:, a, ds(ft * 128, 128)],
                        xt_sb[:, a, ds(sc * QCH, QCH)],
                        start=(a == 0),
                        stop=(a == D_TILES - 1),
                    )
                nc.vector.tensor_copy(qt_sb[:, ft, ds(sc * QCH, QCH)], acc[:])

        def vprep(sc):
            # V^T rows of kvt (64:128) -> seq-major v1 tiles (bf16, ones col)
            for kt in range(4 * sc, 4 * sc + 4):
                tr = ps_acc.tile([128, QCH], F32, tag="acc", name="tracc")
                nc.tensor.transpose(
                    tr[:, 0:64].bitcast(MDT), kvt_sb[64:128, ds(kt * 128, 128)],
                    ident[64:128, 64:128],
                )
                nc.vector.tensor_copy(v1_sb[:, kt, 0:64], tr[:, 0:64].bitcast(MDT))

        kvproj(0, split=True)
        kt2dma(0)
        xchunk(1, 0)
        xchunk(1, 1)
        qproj(0, fts=(0,))

        # ---- deferred-work generators -----------------------------------
        def outproj_st(qc, st, stage_eng="dve"):
            """One 128-row block of the output projection: two psum accs
            staged into a single [128, 1024] store (one HWDGE issue).
            stage_eng picks who drains psum: DVE in steady state, ACT in
            the tail (where the exp stream is finished and ACT is idle)."""
            row0 = qc * QCH + st * 128
            stg = stg_p.tile([128, DIM], BF16, tag="ostg", name="stg")
            for c in range(2):
                acc = ps_acc.tile([128, 512], F32, tag="acc", name="oacc")
                for t in range(2):
                    nc.tensor.matmul(
                        acc[:],
                        ot_sb[:, t, ds(row0, 128)],
                        wo_sb[:, t, ds(c * 512, 512)],
                        start=(t == 0),
                        stop=(t == 1),
                    )
                if stage_eng == "act" and c == 0:
                    nc.scalar.copy(stg[:, ds(c * 512, 512)], acc[:])
                else:
                    nc.vector.tensor_copy(stg[:, ds(c * 512, 512)], acc[:])
                yield
            nc.sync.dma_start(out[ds(row0, 128), :], stg[:])

        def outproj_gen(qc):
            for st in range(QCH // 128):
                yield from outproj_st(qc, st)

        def pv_work(qc, ft, e2t, tail=False):
            """PV + normalize + transpose for phase (qc, ft); yields between
            slices so the caller can interleave it with the next phase.
            Each (query-block, head) accumulation bursts through one acc-pool
            bank (open-close within the burst; one open group per bank) and
            is normalized straight from PSUM, freeing the bank immediately.
            e2t is block-major: block g = 2*kt + h."""
            o_sb = osb_p.tile([128, 4, 128], BF16, tag="osb", name="osb")

            def group_body(h, qt4, split, pool=None):
                pvb = (ps_sc.tile([128, 3, QCH], F32, tag="sc", name="pvsc")[:, 0, :]
                       if pool == "sc" else
                       ps_acc.tile([128, 512], F32, tag="acc", name="pvb"))
                for kt in range(S_TILES):
                    nc.tensor.matmul(
                        pvb[:, 0:65],
                        e2t[:, 2 * kt + h, ds(qt4 * 128, 128)],
                        v1_sb[:, kt, :],
                        start=(kt == 0),
                        stop=(kt == S_TILES - 1),
                    )
                    if split and kt == S_TILES // 2 - 1:
                        yield
                rz = rz_p.tile([128, 1], F32, tag="rz", name="rz")
                nc.vector.reciprocal(rz[:], pvb[:, 64:65])
                nc.vector.tensor_scalar(
                    o_sb[:, qt4, ds(h * 64, 64)], pvb[:, 0:64],
                    rz[:], None, AluOpType.mult,
                )
                if split:
                    yield

            def group(h, qt4):
                for _ in group_body(h, qt4, False):
                    pass

            def group_split(h, qt4):
                yield from group_body(h, qt4, True)

            def transp(qt4, pool):
                tr = (ps_sc.tile([128, 3, QCH], F32, tag="sc", name="trsc")
                      if pool == "sc" else
                      ps_acc.tile([128, 512], F32, tag="acc", name="trx"))
                trv = (tr[:, 0, :] if pool == "sc" else tr[:, :]).bitcast(BF16)
                nc.tensor.transpose(trv[:, 0:128], o_sb[:, qt4, :], identb[:, :])
                nc.vector.tensor_copy(
                    ot_sb[:, ft, ds(qc * QCH + qt4 * 128, 128)], trv[:, 0:128]
                )

            if tail:
                for qt4 in range(4):
                    for h in range(2):
                        # qt0 bursts through the acc banks so they chase the
                        # final exps concurrently with the sc pool's last
                        # score tiles; later blocks use the freed sc pool.
                        for _ in group_body(h, qt4, False,
                                            pool=None if qt4 == 0 else "sc"):
                            pass
                    # chase each block immediately: PE transpose (into the
                    # now-idle score pool) then its output-projection rows,
                    # staged on the idle ACT engine.
                    transp(qt4, "sc")
                    for _ in outproj_st(qc, qt4, stage_eng="act"):
                        pass
                return
            for qt4 in range(4):
                for h in range(2):
                    yield from group_split(h, qt4)
                transp(qt4, "acc")
                yield

        # ---- attention phases -------------------------------------------
        prev_gen = None
        for qc in range(N_QC):
            qsl = ds(qc * QCH, QCH)
            for ft in range(2):
                e2t = e2_p.tile([128, 2 * S_TILES, QCH], BF16, tag="e2t",
                                name="e2t")
                # deferred work interleaved into this phase's kt slots:
                # previous phase's PV chain, this-column qproj, the output
                # projection two phases back (at ft==1), and (first phase)
                # the streaming K/V projections.
                slotmap = {}
                steps = []
                if qc == 0 and ft == 0:
                    # streaming prologue: K/V projections placed just before
                    # the first score tile that needs them, so early scores
                    # are not head-of-line blocked behind later x chunks.
                    def kvstage(c):
                        kvproj(c)
                        kt2dma(c)
                        if c + 1 < N_QC:
                            xchunk(c + 1, 0)
                            xchunk(c + 1, 1)
                        else:
                            nc.sync.dma_start(wo_sb[:, :, :], wo[:, :, :])

                    slotmap = {
                        3: lambda: kvstage(1),
                        5: lambda: vprep(0),
                        6: lambda: vprep(1),
                        7: lambda: kvstage(2),
                        9: lambda: vprep(2),
                        11: lambda: kvstage(3),
                        12: lambda: vprep(3),
                        13: lambda: qproj(0, fts=(1,)),
                        15: lambda: qproj(1, fts=(0,)),
                    }
                elif qc + 1 < N_QC:
                    _qh = {}

                    def qp_half(first, qc=qc, ft=ft, _qh=_qh):
                        sc_n = qc + 1
                        if first:
                            _qh["acc"] = ps_acc.tile(
                                [128, QCH], F32, tag="acc", name="qacc"
                            )
                        acc = _qh["acc"]
                        for a in (range(4) if first else range(4, D_TILES)):
                            nc.tensor.matmul(
                                acc[:],
                                wq_sb[:, a, ds(ft * 128, 128)],
                                xt_sb[:, a, ds(sc_n * QCH, QCH)],
                                start=(a == 0),
                                stop=(a == D_TILES - 1),
                            )
                        if not first:
                            nc.vector.tensor_copy(
                                qt_sb[:, ft, ds(sc_n * QCH, QCH)], acc[:]
                            )

                    steps.append(lambda: qp_half(True))
                    steps.append(lambda: qp_half(False))
                gens = []
                if ft == 1 and qc >= 1:
                    gens.append(outproj_gen(qc - 1))
                if prev_gen is not None:
                    gens.append(prev_gen)
                rr = [0]

                def drain_one():
                    # round-robin across active generators
                    while gens:
                        i = rr[0] % len(gens)
                        try:
                            next(gens[i])
                            rr[0] += 1
                            return
                        except StopIteration:
                            gens.pop(i)

                sc3 = None
                for kt in range(S_TILES):
                    ksl = ds(kt * 128, 128)
                    for h in range(2):
                        g = 2 * kt + h
                        slot = g % 3
                        if slot == 0:
                            sc3 = ps_sc.tile([128, 3, QCH], F32, tag="sc",
                                             name="sc3")
                        if h == 0:
                            nc.tensor.matmul(
                                sc3[:, slot, :], kvt_sb[0:64, ksl],
                                qt_sb[0:64, ft, qsl],
                                start=True, stop=True,
                            )
                        else:
                            nc.tensor.matmul(
                                sc3[:, slot, :], kt2_sb[64:128, ksl],
                                qt_sb[64:128, ft, qsl],
                                start=True, stop=True,
                            )
                        if slot == 2 or g == 2 * S_TILES - 1:
                            n = slot + 1
                            nc.scalar.activation(
                                e2t[:, ds(g - n + 1, n), :], sc3[:, 0:n, :],
                                mybir.ActivationFunctionType.Exp,
                            )
                    if kt in slotmap:
                        slotmap[kt]()
                    elif steps:
                        steps.pop(0)()
                    else:
                        drain_one()
                while gens:
                    drain_one()

                last = (qc == N_QC - 1 and ft == 1)
                prev_gen = pv_work(qc, ft, e2t, tail=last)
        # tail: final phase's PV chain + its output projection
        for _ in prev_gen:
            pass
        if debug_taps:
            nc.sync.dma_start(dbg_qt[:, :, :], qt_sb[:, :, 0:512].bitcast(F32))
            nc.sync.dma_start(dbg_kv[:, :], kvt_sb[:, 0:512].bitcast(F32))
            nc.sync.dma_start(dbg_kt2[:, :], kt2_sb[:, 0:512].bitcast(F32))
            nc.sync.dma_start(dbg_ot[:, :, :], ot_sb[:, :, 0:512])

    return nc


# The neuronx compiler in this environment accepts only ONE sync-wait command
# per instruction; Tile emits instructions with several. Waiting is monotone,
# so hoisting all but the last wait onto same-engine NoOps is equivalent.
_wsctr = [0]


def split_multi_waits(nc):
    n_split = 0
    for f in nc.m.functions:
        for bb in f.blocks:
            insts = bb.instructions
            if not any(
                i.sync_info is not None and len(i.sync_info.on_wait) > 1
                for i in insts
            ):
                continue
            new = []
            for i in insts:
                si = i.sync_info
                if si is not None and len(si.on_wait) > 1:
                    waits = list(si.on_wait)
                    for w in waits[:-1]:
                        _wsctr[0] += 1
                        nop = mybir.InstNoOp(name=f"wsplit_{_wsctr[0]}", ins=[], outs=[])
                        nop.engine = i.engine
                        nop.sync_info = mybir.SyncInfo(on_wait=[w], on_update=[])
                        new.append(nop)
                    i.sync_info = mybir.SyncInfo(
                        on_wait=[waits[-1]], on_update=list(si.on_update)
                    )
                    n_split += 1
                new.append(i)
            bb.instructions = new
    return n_split


def build(use_f32r=True):
    nc = bass.Bass(target_bir_lowering=False)
    build_attention_core(nc, SEQ, use_f32r=use_f32r)
    split_multi_waits(nc)
    return nc


def shard_inputs(x, wq, wk, wv, wo):
    """Full inputs -> per-core in_maps. Core c = (b = c//4, g = c%4)."""
    x = np.asarray(x, np.float32)
    wq = np.asarray(wq, np.float32)
    wk = np.asarray(wk, np.float32)
    wv = np.asarray(wv, np.float32)
    wo = np.asarray(wo, np.float32)
    import ml_dtypes
    bf16 = ml_dtypes.bfloat16
    xTs = [np.ascontiguousarray(x[b].T).astype(bf16) for b in range(BSZ)]

    def tile_pmajor(w, nt):
        # [nt*128, n] -> [128, nt, n] (partition-major, contiguous rows)
        n = w.shape[1]
        return np.ascontiguousarray(
            w.reshape(nt, 128, n).transpose(1, 0, 2)
        ).astype(bf16)

    in_maps = []
    for c in range(N_CORES):
        b, g = c // 4, c % 4
        # fold the 1/sqrt(head_dim) score scaling into wq
        wq_g = tile_pmajor(wq[:, g * FH:(g + 1) * FH] * (1.0 / np.sqrt(HD)), 8)
        wkv_g = tile_pmajor(
            np.concatenate(
                [wk[:, g * HD:(g + 1) * HD], wv[:, g * HD:(g + 1) * HD]], axis=1
            ), 8,
        )
        wo_g = tile_pmajor(wo[g * FH:(g + 1) * FH, :], 2)
        in_maps.append(
            {"xT": xTs[b], "wq": wq_g, "wkv": wkv_g, "wo": wo_g}
        )
    return in_maps


def unshard_output(results):
    """Sum the 4 row-parallel partial outputs per batch."""
    out = np.zeros((BSZ, SEQ, DIM), np.float32)
    for c in range(N_CORES):
        out[c // 4] += np.asarray(results[c]["out"], np.float32)
    return out


_cache = {}


def kernel(x, wq, wk, wv, wo):
    if "nc" not in _cache:
        _cache["nc"] = build()
    nc = _cache["nc"]
    in_maps = shard_inputs(x, wq, wk, wv, wo)
    try:
        res = bass_utils.run_bass_kernel_spmd(
            nc, in_maps, core_ids=list(range(N_CORES))
        )
    except ModuleNotFoundError:
        # BASS_TRACE under an axon client without the NTFF hook module;
        # rerun untraced.
        import os

        os.environ["BASS_NEVER_TRACE"] = "1"
        res = bass_utils.run_bass_kernel_spmd(
            nc, in_maps, core_ids=list(range(N_CORES))
        )
    return unshard_output(res.results)
